# revision 23
# baseline (speedup 1.0000x reference)
"""AFNO2D block-diagonal spectral MLP kernel for 8 Trainium2 NeuronCores.

Math (after simplification of the reference; see reference.py):
  H = W = 128, nb = 8, bs = 96; kept == W so mode truncation is a no-op and
  the imaginary output o2i is discarded.  With halves folded into weights:
    o1r = relu(Xk@(w10/2) + Xn@(w11/2)... ) etc.  For a mirror pair of site
  tiles T, T~ (Xk/Xn swap), define S = x(T)+x(T~), D = x(T)-x(T~) (host):
    o1r(T)  = relu(P + R + b1r)      P = S@(w10/2), R = D@(w11/2)
    o1r(T~) = relu(P - R + b1r)
    o1i(T)  = relu(Q + b1i)          Q = D@D1h, D1h = (w10-w11)/2
    o1i(T~) = relu(-Q + b1i)
    z(T)    = o1r@A2h + o1i@D2h + b2r     (A2h/D2h = (w2[0]±w2[1])/2)
    out     = x + softshrink(z, 0.01)     (residual added on host)

Device mapping per (group, block) unit (1024 sites = 512 mirror pairs):
  5 fp8 DoubleRow matmuls (K pairs (S[p], D[p]) + a ones-row riding biases):
    A = P+R+b1r | C = Q+b1i  -> one adjacent psum pair, drained by ONE
    ScalarE relu into the fp8 interleaved (o1r|o1i) layer-2 moving tile.
    B = P-R+b1r  (ScalarE relu), o1i~ from C (VectorE/ScalarE), then
    zT/zTn DoubleRow matmuls re-use the A|C psum banks, drained by a
    custom DVE softshrink op (x - clamp(x, -l, l)) straight to fp8.
  The 16 self-mirror sites (i,j in {0,64}) are computed on the host; one
  duplicated pair pads each core's 4095 pairs to 8 groups x 512 columns.
"""

import numpy as np
import ml_dtypes

import concourse.bass as bass
import concourse.mybir as mybir
from concourse import bacc
from concourse.tile import TileContext
from concourse import bass_utils
from concourse import dve_ops as _dve_ops
from concourse.dve_spec import Spec, Src0, C0, C1, maxx, minn, lower as _dve_lower
from concourse.dve_uop import DveOpSpec
from concourse.bass_utils import dve_ver_for

FP8 = mybir.dt.float8e4
F32 = mybir.dt.float32
AF = mybir.ActivationFunctionType
ALU = mybir.AluOpType
NPF8 = ml_dtypes.float8_e4m3   # TRN FP8_EXP4-compatible (max 240, has inf)

B, N, C = 4, 16384, 768
H = W = 128
NB, BS = 8, 96
P = BS + 1                     # 96 channels + ones row (bias riding)
LAM = 0.01
NCORES = 8
SITES = B * N                  # 65536
TILE = 512                     # mirror pairs per unit
NGRP = 8                       # groups per core; NGRP*TILE = 4096 pair cols
PAIRS = 4095                   # real pairs per core (+1 duplicated pad col)
WPAD = 112                     # weight free-dim pad so pair stride % 16 == 0
ACT_I_BLOCKS = (0, 4)          # blocks whose o1i~ readout runs on ScalarE
MODE = "reg9"                  # "dr5" DoubleRow 5-MM | "reg9" regular-fp8 9-MM

_cache = {}


def _register_softshrink():
    name = "SOFTSHRINK_ANT"
    for op in _dve_ops.OPS:
        if op.name == name:
            return op
    spec = Spec(
        body=Src0 - minn(maxx(Src0, C0), C1),
        reference=lambda in0, in1, s0, s1, imm2: (
            in0.astype(np.float32) - np.clip(in0.astype(np.float32), s0, s1)
        ),
    )
    row = max(_dve_ops._SUB_OPCODE_FOR_NAME.values()) + 1
    assert row < 0x20
    _dve_ops._SUB_OPCODE_FOR_NAME[name] = row
    ver = dve_ver_for("TRN2")
    sha = DveOpSpec(
        name=name, opcode=row, uops=_dve_lower(spec, ver=ver), rd1_en=False
    ).sha(ver)
    op = _dve_ops.DveOp(name, spec, subdim=False, uops_sha={ver: sha})
    _dve_ops.OPS.append(op)
    _dve_ops.CUSTOM_DVE_SPECS[name] = spec
    return op


SOFTSHRINK = _register_softshrink()


def _build():
    nc = bacc.Bacc("TRN2", target_bir_lowering=False)

    NW = 5 * 2 if MODE == "dr5" else 7
    xd = nc.dram_tensor("x", [NGRP, P, NB * 2 * TILE], FP8, kind="ExternalInput")
    wd = nc.dram_tensor("w", [P, NB * NW * WPAD], FP8, kind="ExternalInput")
    bd = nc.dram_tensor("b", [P, NB], F32, kind="ExternalInput")
    outd = nc.dram_tensor("out", [NGRP, BS, NB * 2 * TILE], FP8,
                          kind="ExternalOutput")

    DR = mybir.MatmulPerfMode.DoubleRow
    A_, B_, C_, ZT_, ZN_ = range(5)

    with TileContext(nc) as tc:
        with (
            tc.tile_pool(name="consts", bufs=1) as consts,
            tc.tile_pool(name="io", bufs=NGRP) as io_pool,
            tc.tile_pool(name="outp", bufs=2) as out_pool,
            tc.tile_pool(name="o1", bufs=3) as o1_pool,
            tc.tile_pool(name="psac", bufs=2, space="PSUM") as psac_pool,
            tc.tile_pool(name="psz", bufs=1, space="PSUM") as psz_pool,
        ):
            wsb = consts.tile([P, NB * NW, WPAD], FP8)
            wflat = wsb.rearrange("p a b -> p (a b)")
            sl0 = 2 * NW * WPAD        # first two blocks' weights first
            bsb = consts.tile([P, NB], F32)

            def wAP(n, s, m):
                base = (n * 5 + s) * 2
                return wsb[:, base:base + 2, 0:m]

            def wR(n, s, m):
                return wsb[:, n * 7 + s, 0:m]

            def bAP(n):
                return bsb[:, n:n + 1]

            group_tiles = {}

            # All input DMAs ride the gpsimd queue, but their *emission* is
            # interleaved into the unit loop: an MM only waits on the queue
            # sem count at its emission point, so late-emitted prefetches
            # don't gate early compute.
            def _dma_sd(j, k, split=2):
                if j not in group_tiles:
                    group_tiles[j] = io_pool.tile([P, NB * 2, TILE], FP8,
                                                  tag="sd", name=f"sd{j}")
                sdf = group_tiles[j].rearrange("p a b -> p (a b)")
                step = NB * 2 * TILE // split
                nc.gpsimd.dma_start(sdf[:, k * step:(k + 1) * step],
                                    xd[j, :, k * step:(k + 1) * step])

            dma_sched = {
                0: [lambda: nc.gpsimd.dma_start(wflat[:, 0:sl0], wd[:, 0:sl0]),
                    lambda: _dma_sd(0, 0, 4),
                    lambda: nc.gpsimd.dma_start(bsb[:], bd[:])],
                1: [lambda: nc.gpsimd.dma_start(wflat[:, sl0:], wd[:, sl0:]),
                    lambda: _dma_sd(0, 1, 4)],
                2: [lambda: _dma_sd(0, 2, 4)],
                3: [lambda: _dma_sd(0, 3, 4)],
            }
            for j in range(1, NGRP):
                dma_sched[2 + 2 * j] = [lambda j=j: _dma_sd(j, 0)]
                dma_sched[3 + 2 * j] = [lambda j=j: _dma_sd(j, 1)]

            out_tiles = {}

            def get_out(j):
                out_tiles[j] = out_pool.tile([BS, NB, 2 * TILE], FP8,
                                             tag="out", name=f"out{j}")

            def stage1(j, n):
                sd = group_tiles[j]
                # one psum tile = A | C | B adjacent banks -> single relu drain
                ps = psac_pool.tile([P, 3 * TILE], F32, tag="acb")
                S = sd[:, 2 * n, :]
                D = sd[:, 2 * n + 1, :]
                nc.tensor.matmul(ps[:, 0:TILE], wR(n, 0, P), S,
                                 start=True, stop=False)
                nc.tensor.matmul(ps[:, 0:TILE], wR(n, 1, P), D,
                                 start=False, stop=True)
                nc.tensor.matmul(ps[:, 2 * TILE:3 * TILE], wR(n, 0, P), S,
                                 start=True, stop=False)
                nc.tensor.matmul(ps[:, 2 * TILE:3 * TILE], wR(n, 2, P), D,
                                 start=False, stop=True)
                nc.tensor.matmul(ps[:, TILE:2 * TILE], wR(n, 3, P), D,
                                 start=True, stop=True)
                o1T = o1_pool.tile([P, 3, TILE], FP8, tag="t")
                o1N = o1_pool.tile([P, TILE], FP8, tag="tn")
                # o1r(T) | o1i(T) | o1r(T~) in ONE drain (biases already in)
                nc.scalar.activation(o1T.rearrange("p a b -> p (a b)"), ps,
                                     AF.Relu)
                if n in ACT_I_BLOCKS:
                    # +o1i(T~) = relu(-C + 2*b1i); pairs with +D2h weights
                    nc.scalar.activation(o1N, ps[:, TILE:2 * TILE],
                                         AF.Relu, bias=bAP(n), scale=-1.0)
                else:
                    # -o1i(T~) = min(C - 2*b1i, 0); pairs with -D2h weights
                    nc.vector.tensor_scalar(o1N, ps[:, TILE:2 * TILE],
                                            bAP(n), 0.0, ALU.subtract, ALU.min)
                return j, n, o1T, o1N

            def stage2(j, n, o1T, o1N):
                out_t = out_tiles[j]
                pz = psz_pool.tile([BS, 2 * TILE], F32, tag="z")
                nc.tensor.matmul(pz[:, 0:TILE], wR(n, 4, BS),
                                 o1T[:, 0, :], start=True, stop=False)
                nc.tensor.matmul(pz[:, 0:TILE], wR(n, 5, BS),
                                 o1T[:, 1, :], start=False, stop=True)
                nc.tensor.matmul(pz[:, TILE:2 * TILE], wR(n, 4, BS),
                                 o1T[:, 2, :], start=True, stop=False)
                nc.tensor.matmul(pz[:, TILE:2 * TILE], wR(n, 6, BS),
                                 o1N, start=False, stop=True)
                # split softshrink drain: zT's bank frees one op earlier
                nc.vector._custom_dve(SOFTSHRINK, out=out_t[:, n, 0:TILE],
                                      in0=pz[:, 0:TILE], s0=-LAM, s1=LAM)
                nc.vector._custom_dve(SOFTSHRINK, out=out_t[:, n, TILE:2 * TILE],
                                      in0=pz[:, TILE:2 * TILE], s0=-LAM, s1=LAM)
                lo = n * 2 * TILE
                nc.sync.dma_start(outd[j, :, lo:lo + 2 * TILE], out_t[:, n, :])

            units = [(j, n) for j in range(NGRP) for n in range(NB)]
            pend = []          # 2-unit skew: L2 of unit k issues after L1(k+2)
            for k, (j, n) in enumerate(units):
                for fn in dma_sched.get(k, ()):
                    fn()
                if n == 0:
                    get_out(j)
                pend.append(stage1(j, n))
                if len(pend) > 2:
                    stage2(*pend.pop(0))
            for s in pend:
                stage2(*s)

    nc.finalize()
    return nc


def _site_order():
    idx = np.arange(SITES)
    b = idx // N
    r = idx % N
    i, jj = r // W, r % W
    mi = b * N + ((-i) % H) * W + ((-jj) % W)
    firsts = idx[idx < mi]                    # 32760 = 8 * 4095
    fixed = idx[idx == mi]                    # 16 self-mirror sites
    F = np.empty((NCORES, NGRP * TILE), dtype=np.int64)
    for c in range(NCORES):
        fc = firsts[c * PAIRS:(c + 1) * PAIRS]
        F[c, :PAIRS] = fc
        F[c, PAIRS:] = fc[0]                  # pad col: duplicated pair
    M = mi[F]
    return F, M, fixed


def _host_prep(x, w1, b1, w2, b2):
    if "order" not in _cache:
        _cache["order"] = _site_order()
    F, M, fixed = _cache["order"]

    xf = np.ascontiguousarray(x.reshape(SITES, C))
    u = xf[F.reshape(-1)].reshape(NCORES, NGRP, TILE, NB, BS)
    v = xf[M.reshape(-1)].reshape(NCORES, NGRP, TILE, NB, BS)
    S = u + v
    Dd = u - v
    sd = np.empty((NCORES, NGRP, P, NB, 2, TILE), dtype=NPF8)
    sd[:, :, :BS, :, 0, :] = S.transpose(0, 1, 4, 3, 2).astype(NPF8)
    sd[:, :, :BS, :, 1, :] = Dd.transpose(0, 1, 4, 3, 2).astype(NPF8)
    sd[:, :, BS, :, :, :] = np.float32(1.0)

    w10h = w1[0] * 0.5
    w11h = w1[1] * 0.5
    D1h = (w1[0] - w1[1]) * 0.5
    A2h = (w2[0] + w2[1]) * 0.5
    D2h = (w2[0] - w2[1]) * 0.5
    wT = lambda m: m.transpose(1, 0, 2)       # [NB,in,out] -> [in,NB,out]
    sgn = np.where(np.isin(np.arange(NB), ACT_I_BLOCKS), 1.0, -1.0)
    if MODE == "dr5":
        wpack = np.zeros((P, NB, 5, 2, WPAD), dtype=np.float32)
        wpack[:BS, :, A_, 0, :BS] = wT(w10h)
        wpack[BS, :, A_, 0, :BS] = b1[0] * 0.5
        wpack[BS, :, A_, 0, BS] = 1.0
        wpack[:BS, :, A_, 1, :BS] = wT(w11h)
        wpack[:, :, B_, 0, :] = wpack[:, :, A_, 0, :]
        wpack[:BS, :, B_, 1, :BS] = wT(-w11h)
        wpack[BS, :, C_, 0, :BS] = b1[1] * 0.5
        wpack[BS, :, C_, 0, BS] = 1.0
        wpack[:BS, :, C_, 1, :BS] = wT(D1h)
        wpack[:BS, :, ZT_, 0, :BS] = wT(A2h)
        wpack[BS, :, ZT_, 0, :BS] = b2[0] * 0.5
        wpack[:BS, :, ZT_, 1, :BS] = wT(D2h)
        wpack[:, :, ZN_, 0, :] = wpack[:, :, ZT_, 0, :]
        wpack[:BS, :, ZN_, 1, :BS] = wT(D2h * sgn[:, None, None])
        wpack8 = wpack.reshape(P, NB * 5 * 2 * WPAD).astype(NPF8)
    else:
        wpack = np.zeros((P, NB, 7, WPAD), dtype=np.float32)
        wpack[:BS, :, 0, :BS] = wT(w10h)      # A0: + b1r row + const col
        wpack[BS, :, 0, :BS] = b1[0] * 0.5
        wpack[BS, :, 0, BS] = 1.0
        wpack[:BS, :, 1, :BS] = wT(w11h)      # A1
        wpack[:BS, :, 2, :BS] = wT(-w11h)     # B1
        wpack[:BS, :, 3, :BS] = wT(D1h)       # C1: + b1i row + const col
        wpack[BS, :, 3, :BS] = b1[1] * 0.5
        wpack[BS, :, 3, BS] = 1.0
        wpack[:BS, :, 4, :BS] = wT(A2h)       # Z0: + b2r row
        wpack[BS, :, 4, :BS] = b2[0] * 0.5
        wpack[:BS, :, 5, :BS] = wT(D2h)       # Z1
        wpack[:BS, :, 6, :BS] = wT(D2h * sgn[:, None, None])  # Z1n
        wpack8 = wpack.reshape(P, NB * 7 * WPAD).astype(NPF8)

    bpack = np.empty((P, NB), dtype=np.float32)
    bpack[:BS] = b1[1].T                      # 2*b1i
    bpack[BS] = 0.5
    in_maps = []
    for c in range(NCORES):
        in_maps.append({
            "x": np.ascontiguousarray(sd[c].reshape(NGRP, P, NB * 2 * TILE)),
            "w": wpack8,
            "b": bpack,
        })
    return in_maps


A_, B_, C_, ZT_, ZN_ = range(5)


def _fixed_out(xf, w1, b1, w2, b2, fixed):
    xs = xf[fixed].reshape(len(fixed), NB, BS)
    o1r = np.maximum(np.einsum("knp,npq->knq", xs, w1[0]) + 0.5 * b1[0], 0.0)
    o1i = np.maximum(0.5 * b1[1], 0.0)[None]
    A2h = (w2[0] + w2[1]) * 0.5
    D2h = (w2[0] - w2[1]) * 0.5
    z = (np.einsum("knp,npq->knq", o1r, A2h)
         + np.einsum("knp,npq->knq", np.broadcast_to(o1i, o1r.shape), D2h)
         + 0.5 * b2[0])
    return (z - np.clip(z, -LAM, LAM)).reshape(len(fixed), C)


def _assemble(results, x, w1, b1, w2, b2):
    F, M, fixed = _cache["order"]
    full = np.zeros((SITES, C), dtype=np.float32)
    for c in range(NCORES):
        o = results[c]["out"].reshape(NGRP, BS, NB, 2, TILE)
        t = o[:, :, :, 0, :].transpose(0, 3, 2, 1).reshape(NGRP * TILE, C)
        tn = o[:, :, :, 1, :].transpose(0, 3, 2, 1).reshape(NGRP * TILE, C)
        full[F[c]] = t.astype(np.float32)
        full[M[c]] = tn.astype(np.float32)
    xf = x.reshape(SITES, C)
    full[fixed] = _fixed_out(xf, w1, b1, w2, b2, fixed)
    return (xf + full).reshape(B, N, C)


def _run(x, w1, b1, w2, b2, trace=False):
    if "nc" not in _cache:
        _cache["nc"] = _build()
    nc = _cache["nc"]
    in_maps = _host_prep(x, w1, b1, w2, b2)
    res = bass_utils.run_bass_kernel_spmd(
        nc, in_maps, core_ids=list(range(NCORES)), trace=trace)
    return _assemble(res.results, x, w1, b1, w2, b2), res


def kernel(x, w1, b1, w2, b2):
    out, _ = _run(x, w1, b1, w2, b2, trace=False)
    return out
